# revision 1
# baseline (speedup 1.0000x reference)
"""Sliding-window causal attention (RoPE + GQA) Trainium2 Bass kernel.

Problem: B=2, S=2048, H=32 q-heads, KVH=8 kv-heads, D=64, window=256 (left,
causal right), RoPE base 10000.  8 NeuronCores, data+head parallel:
16 units of (batch, kv-group) -> 2 units per core.

Self-contained: hardcodes shapes; builds per-core numpy shards, runs one
SPMD Bass program on cores 0-7 via run_bass_kernel_spmd, re-assembles the
full [2, 2048, 2048] output.
"""

import numpy as np

import concourse.bass as bass
import concourse.bacc as bacc
import concourse.mybir as mybir
import concourse.tile as tile
from concourse.bass_utils import run_bass_kernel_spmd

F32 = mybir.dt.float32
F32R = mybir.dt.float32r
BF16 = mybir.dt.bfloat16

B = 2
S = 2048
H = 32
KVH = 8
D = 64
WIN = 256
NREP = H // KVH          # 4 q heads per kv head
N_CORES = 8
UNITS_PER_CORE = 2       # 16 (b, g) units / 8 cores
QB = S // 128            # 16 query row-blocks
ROPE_BASE = 10000.0
MASK_VAL = -1e9

# head slot order inside the 512-wide score strips (natural order)
SLOT_TO_HEAD = [0, 1, 2, 3]


def _blocks(qb):
    """[(kb, kind)] for query block qb; kind: 0=UPPER tri, 1=full, 2=DIAG tri."""
    if qb == 0:
        return [(0, 2)]
    if qb == 1:
        return [(0, 1), (1, 2)]
    return [(qb - 2, 0), (qb - 1, 1), (qb, 2)]


def _emit_rope(nc, pools, x1, x2, cos, sin, sinn, tmp_shape, tag):
    """In-place RoPE halves: x1 <- x1*c - x2*s ; x2 <- x2*c + x1*s.

    All tensor_tensor ops (the TensorScalarPtr instruction format runs out of
    sync-wait slots in walrus codegen); sinn is a pre-negated sin table.
    """
    pool = pools["tmp"]
    t = pool.tile(tmp_shape, F32, tag=f"{tag}_t")
    u = pool.tile(tmp_shape, F32, tag=f"{tag}_u")
    v = pool.tile(tmp_shape, F32, tag=f"{tag}_v")
    w = pool.tile(tmp_shape, F32, tag=f"{tag}_w")
    nc.gpsimd.tensor_mul(t[:], x2, sinn)   # t = -x2 * s   (gpsimd: otherwise idle)
    nc.gpsimd.tensor_mul(v[:], x2, cos)    # v = x2 * c
    nc.vector.tensor_mul(u[:], x1, cos)    # u = x1 * c
    nc.vector.tensor_mul(w[:], x1, sin)    # w = x1 * s
    nc.vector.tensor_add(x1, u[:], t[:])
    nc.vector.tensor_add(x2, v[:], w[:])


def build_program():
    nc = bacc.Bacc("TRN2", target_bir_lowering=False, debug=False)

    q_d = nc.dram_tensor("q", [UNITS_PER_CORE, S, NREP * D], F32, kind="ExternalInput").ap()
    k_d = nc.dram_tensor("k", [UNITS_PER_CORE, S, D], F32, kind="ExternalInput").ap()
    v_d = nc.dram_tensor("v", [UNITS_PER_CORE, S, D], F32, kind="ExternalInput").ap()
    cos_d = nc.dram_tensor("cos_h", [S, D // 2], F32, kind="ExternalInput").ap()
    sin_d = nc.dram_tensor("sin_h", [S, D // 2], F32, kind="ExternalInput").ap()
    id_d = nc.dram_tensor("ident", [128, 128], F32, kind="ExternalInput").ap()
    ma_d = nc.dram_tensor("maskA4", [128, 512], F32, kind="ExternalInput").ap()
    mb_d = nc.dram_tensor("maskB4", [128, 512], F32, kind="ExternalInput").ap()
    out_d = nc.dram_tensor("out", [UNITS_PER_CORE, S, NREP * D], F32, kind="ExternalOutput").ap()

    with tile.TileContext(nc) as tc:
        with (
            tc.tile_pool(name="const", bufs=1) as constp,
            tc.tile_pool(name="unit", bufs=2) as unitp,
            tc.tile_pool(name="tmp", bufs=1) as tmpp,
            tc.tile_pool(name="qt", bufs=3) as qtp,
            tc.tile_pool(name="pt", bufs=2) as ptp,
            tc.tile_pool(name="outs", bufs=3) as outsp,
            tc.tile_pool(name="trp", bufs=1, space="PSUM") as trpp,
            tc.tile_pool(name="stap", bufs=2, space="PSUM") as stap,
            tc.tile_pool(name="stbp", bufs=2, space="PSUM") as stbp,
            tc.tile_pool(name="ovpo", bufs=1, space="PSUM") as ovpop,
        ):
            pools = {"tmp": tmpp}

            ident = constp.tile([128, 128], F32)
            identb = constp.tile([128, 128], BF16)
            identr = constp.tile([128, 128], F32R)
            maskA4 = constp.tile([128, 512], F32)
            maskA4r = constp.tile([128, 512], F32R)
            cosL = constp.tile([128, QB, 32], F32)
            sinL = constp.tile([128, QB, 32], F32)
            sinLn = constp.tile([128, QB, 32], F32)
            cos4 = constp.tile([128, QB, NREP, 32], F32)
            sin4 = constp.tile([128, QB, NREP, 32], F32)
            sin4n = constp.tile([128, QB, NREP, 32], F32)

            nc.sync.dma_start(out=ident[:], in_=id_d)
            nc.sync.dma_start(out=maskA4[:], in_=ma_d)
            nc.sync.dma_start(out=cosL[:], in_=cos_d.rearrange("(qb r) j -> r qb j", r=128))
            nc.sync.dma_start(out=sinL[:], in_=sin_d.rearrange("(qb r) j -> r qb j", r=128))
            nc.gpsimd.tensor_copy(identb[:], ident[:])
            nc.gpsimd.tensor_copy(identr[:], ident[:])
            nc.gpsimd.tensor_copy(maskA4r[:], maskA4[:])
            nc.vector.tensor_scalar_mul(sinLn[:], sinL[:], -1.0)
            for rep in range(NREP):
                nc.gpsimd.tensor_copy(cos4[:, :, rep, :], cosL[:])
                nc.gpsimd.tensor_copy(sin4[:, :, rep, :], sinL[:])
                nc.gpsimd.tensor_copy(sin4n[:, :, rep, :], sinLn[:])

            for u in range(UNITS_PER_CORE):
                # ---------------- phase A: per-unit K/V/Q prep ----------------
                qnat = unitp.tile([128, QB, NREP * D], F32, tag="qnat")
                knat = unitp.tile([128, QB, D], F32, tag="knat")
                vraw = unitp.tile([128, QB, D], F32, tag="vraw")
                vaug = unitp.tile([128, QB, D + 1], BF16, tag="vaug")
                kt = unitp.tile([64, QB, 128], F32R, tag="kt")

                nc.sync.dma_start(out=qnat[:], in_=q_d[u].rearrange("(qb r) c -> r qb c", r=128))
                nc.sync.dma_start(out=knat[:], in_=k_d[u].rearrange("(qb r) c -> r qb c", r=128))
                nc.sync.dma_start(out=vraw[:], in_=v_d[u].rearrange("(kb p) c -> p kb c", p=128))
                nc.gpsimd.tensor_copy(vaug[:, :, 0:D], vraw[:])
                nc.gpsimd.memset(vaug[:, :, D : D + 1], 1.0)

                # RoPE K in natural layout [128, 16, 64]
                _emit_rope(
                    nc, pools,
                    knat[:, :, 0:32], knat[:, :, 32:64],
                    cosL[:], sinL[:], sinLn[:],
                    [128, QB, 32], "k",
                )
                # RoPE Q in natural layout [128, 16, 4, 64]
                qr = qnat.rearrange("p qb (h c) -> p qb h c", h=NREP)
                _emit_rope(
                    nc, pools,
                    qr[:, :, :, 0:32], qr[:, :, :, 32:64],
                    cos4[:], sin4[:], sin4n[:],
                    [128, QB, NREP, 32], "q",
                )

                # K transposes: single [128,64] -> [64,128] per key block,
                # grouped 4 per PSUM bank so one wide DVE copy drains them.
                # NOTE: matmul operands at partition base 64 fail on HW, so
                # everything stays at base 0.
                for k4 in range(QB // 4):
                    ps = trpp.tile([64, 4, 128], F32, tag="trp")
                    for j in range(4):
                        nc.tensor.matmul(
                            ps[:, j, :], knat[:, k4 * 4 + j, :], ident[:],
                            is_transpose=True, start=(j == 0), stop=(j == 3),
                        )
                    nc.vector.tensor_copy(kt[:, k4 * 4 : k4 * 4 + 4, :], ps[:])

                # ---------------- phase B: per query-block attention ----------------
                for qb in range(QB):
                    blocks = _blocks(qb)
                    nb = len(blocks)

                    # Q^T for 4 heads: single transposes into one PSUM bank,
                    # drained by one wide DVE copy; all at base 0.
                    qt = qtp.tile([64, NREP, 128], F32R, tag="qt")
                    ps = trpp.tile([64, NREP, 128], F32, tag="trp")
                    for h in range(NREP):
                        nc.tensor.matmul(
                            ps[:, h, :], qnat[:, qb, h * 64 : (h + 1) * 64], ident[:],
                            is_transpose=True, start=(h == 0), stop=(h == NREP - 1),
                        )
                    nc.vector.tensor_copy(qt[:], ps[:])

                    # scores^T in PSUM, split in two double-buffered pools so
                    # consecutive qb iterations overlap: strips 0-1 in sta,
                    # strip 2 (diag) in stb.
                    sta = stap.tile([128, 2 * 512], F32, tag="sta")
                    stb = stbp.tile([128, 512], F32, tag="stb")

                    def st_slice(bi):
                        if bi == nb - 1:  # diag strip always in stb
                            return stb[:]
                        return sta[:, bi * 512 : (bi + 1) * 512]

                    for bi, (kb, kind) in enumerate(blocks):
                        nc.tensor.matmul(
                            st_slice(bi),
                            kt[:, kb, :],
                            qt[:].rearrange("p h s -> p (h s)"),
                            start=True, stop=(kind != 0),
                        )
                        if kind == 0:  # UPPER: additive mask on PE
                            nc.tensor.matmul(
                                st_slice(bi),
                                identr[:],
                                maskA4r[:],
                                start=False, stop=True,
                            )

                    # probs^T = exp(scale * scores^T) -> SBUF (one ACT op per pool)
                    pt = ptp.tile([128, 3 * 512], BF16, tag="pt")
                    na = nb - 1
                    if na > 0:
                        nc.scalar.activation(
                            pt[:, 0 : na * 512],
                            sta[:, 0 : na * 512],
                            mybir.ActivationFunctionType.Exp,
                            scale=0.125,
                        )
                    nc.scalar.activation(
                        pt[:, na * 512 : nb * 512],
                        stb[:],
                        mybir.ActivationFunctionType.Exp,
                        scale=0.125,
                    )
                    # zero masked-out probs (gpsimd; pattern per 128-col head
                    # sub-block, stride-0 over the 4 heads):
                    for bi, (kb, kind) in enumerate(blocks):
                        if kind != 2:
                            continue
                        strip = pt[:, bi * 512 : (bi + 1) * 512].rearrange(
                            "p (h r) -> p h r", h=NREP
                        )
                        # DIAG: keep c <= r
                        nc.gpsimd.affine_select(
                            out=strip, in_=strip,
                            compare_op=mybir.AluOpType.is_ge,
                            fill=0.0, base=0,
                            pattern=[[0, NREP], [1, 128]],
                            channel_multiplier=-1,
                        )

                    # PV: out^T[65, 512] accumulated over key blocks
                    ov = ovpop.tile([65, 512], F32, tag="ovpo")
                    for bi, (kb, kind) in enumerate(blocks):
                        nc.tensor.matmul(
                            ov[:],
                            vaug[:, kb, :],
                            pt[:, bi * 512 : (bi + 1) * 512],
                            start=(bi == 0), stop=(bi == nb - 1),
                        )
                    ovs = outsp.tile([65, 512], BF16, tag="ovs")
                    nc.scalar.copy(ovs[:], ov[:])

                    # transpose each head strip back to [128 q, 65] and normalize
                    po = ovpop.tile([128, 4 * 66], BF16, tag="ovpo")  # 66: keep 4B-aligned matmul outs
                    por = po.rearrange("p (i c) -> p i c", c=66)
                    for i in range(4):
                        # po is one PSUM bank: single accumulation group across
                        # the 4 disjoint transposes (one start, one stop).
                        nc.tensor.matmul(
                            por[:, i, 0:65],
                            ovs[:, i * 128 : (i + 1) * 128],
                            identb[0:65, 0:65],
                            is_transpose=True,
                            start=(i == 0), stop=(i == 3),
                        )
                    rsum = outsp.tile([128, 4], F32, tag="rsum")
                    nc.vector.tensor_copy(rsum[:], por[:, :, 64])
                    recip = outsp.tile([128, 4], F32, tag="recip")
                    nc.vector.reciprocal_approx_fast(recip[:], rsum[:])
                    osb = outsp.tile([128, 256], F32, tag="osb")
                    osbr = osb.rearrange("p (i c) -> p i c", c=64)
                    recip_b = recip[:].rearrange("p (i o) -> p i o", o=1).broadcast_to(
                        [128, 4, 64]
                    )  # stride-0 along the last dim
                    nc.vector.tensor_tensor(
                        osbr[:], por[:, :, 0:64], recip_b,
                        op=mybir.AluOpType.mult,
                    )
                    nc.sync.dma_start(
                        out=out_d[u, qb * 128 : (qb + 1) * 128, :], in_=osb[:]
                    )
    nc.compile()
    return nc


def _host_tables():
    inv_freq = 1.0 / (ROPE_BASE ** (np.arange(0, D, 2, dtype=np.float32) / D))
    pos = np.arange(S, dtype=np.float32)
    freqs = pos[:, None] * inv_freq[None, :]                  # [S, 32]
    cos_h = np.cos(freqs).astype(np.float32)
    sin_h = np.sin(freqs).astype(np.float32)
    ident = np.eye(128, dtype=np.float32)
    c = np.arange(128)[:, None]
    r = np.arange(128)[None, :]
    maskA = np.where(c >= r, 0.0, MASK_VAL).astype(np.float32)   # UPPER: valid c>=r
    maskB = np.where(c <= r, 0.0, MASK_VAL).astype(np.float32)   # DIAG:  valid c<=r
    return cos_h, sin_h, ident, np.tile(maskA, (1, 4)), np.tile(maskB, (1, 4))


_NC_CACHE = None


def _get_nc():
    global _NC_CACHE
    if _NC_CACHE is None:
        _NC_CACHE = build_program()
    return _NC_CACHE


def _make_in_maps(query_states, key_states, value_states):
    q = np.asarray(query_states, dtype=np.float32)
    k = np.asarray(key_states, dtype=np.float32)
    v = np.asarray(value_states, dtype=np.float32)
    cos_h, sin_h, ident, ma4, mb4 = _host_tables()
    in_maps = []
    for c in range(N_CORES):
        qs, ks, vs = [], [], []
        for uu in range(UNITS_PER_CORE):
            unit = c * UNITS_PER_CORE + uu
            b, g = divmod(unit, KVH)
            qs.append(q[b, :, g * NREP * D : (g + 1) * NREP * D])
            ks.append(k[b, :, g * D : (g + 1) * D])
            vs.append(v[b, :, g * D : (g + 1) * D])
        in_maps.append(
            {
                "q": np.stack(qs), "k": np.stack(ks), "v": np.stack(vs),
                "cos_h": cos_h, "sin_h": sin_h, "ident": ident,
                "maskA4": ma4, "maskB4": mb4,
            }
        )
    return in_maps


def run(query_states, key_states, value_states, **run_kwargs):
    nc = _get_nc()
    in_maps = _make_in_maps(query_states, key_states, value_states)
    res = run_bass_kernel_spmd(nc, in_maps, list(range(N_CORES)), **run_kwargs)
    out = np.empty((B, S, H * D), dtype=np.float32)
    for c in range(N_CORES):
        o = res.results[c]["out"]
        for uu in range(UNITS_PER_CORE):
            unit = c * UNITS_PER_CORE + uu
            b, g = divmod(unit, KVH)
            out[b, :, g * NREP * D : (g + 1) * NREP * D] = o[uu]
    return out, res


def kernel(query_states, key_states, value_states):
    out, _ = run(query_states, key_states, value_states)
    return out

